# revision 10
# baseline (speedup 1.0000x reference)
"""Trainium2 Bass kernel for nn_DistillMoE (noisy top-2 MoE, 8 experts, B=131072, D=512).

Strategy (v3, host-sorted sparse dispatch, fully fused):
- 8-way data parallel over NeuronCores. The host groups tokens by their
  top-2 expert PAIR (28 unordered pairs) and round-robins each pair's
  tokens across cores, so all cores share ONE static slot schedule.
  Each core's input shard is laid out in slot order (group regions
  128-aligned, zero padding rows) — so the device needs NO gather or
  scatter at all; the expert pair for each 128-token tile is baked into
  the traced program.
- Device, per 128-token tile: PE-transpose (fp32, exact) -> fp32 router
  matmuls -> noisy-top2 softmax gating on DVE/ACT (all routing math on
  device) -> TWO float32r expert GEMMs (full PE rate, ~1e-4) -> gate
  combine on ACT+DVE -> stream out. Host scatters rows back to the
  original token order.

kernel(**inputs) takes FULL inputs, returns (updates, gating_output).
"""
import sys
import numpy as np

sys.path.insert(0, "/opt/trn_rl_repo")

from concourse import bacc, mybir  # noqa: E402
from concourse.tile import TileContext  # noqa: E402
from concourse.bass_utils import run_bass_kernel_spmd  # noqa: E402
from concourse.masks import make_identity  # noqa: E402

N_CORES = 8
B_FULL = 131072
D = 512
E = 8
ST = 4                          # tiles per super-tile
SLACK = 16                      # per-group capacity slack (tokens)

f32 = mybir.dt.float32
f32r = mybir.dt.float32r
AF = mybir.ActivationFunctionType
OP = mybir.AluOpType

_CACHE = {}


def _build(schedule, use_bias, use_rbias):
    """schedule: tuple of (a, b) expert pairs, one per 128-token tile."""
    key = (schedule, use_bias, use_rbias)
    if key in _CACHE:
        return _CACHE[key]
    NT = len(schedule)
    assert NT % ST == 0
    NS = NT * 128
    n_st = NT // ST
    nc = bacc.Bacc("TRN2", target_bir_lowering=False, debug=False, num_devices=N_CORES)

    d_xt = nc.dram_tensor("xt", [128, NS // 512, ST * 4 * 128], f32, kind="ExternalInput")
    d_noise = nc.dram_tensor("noise", [NS, E], f32, kind="ExternalInput")
    d_wrn = nc.dram_tensor("wrn", [D, 2 * E], f32, kind="ExternalInput")
    d_brn = nc.dram_tensor("brn", [1, 2 * E], f32, kind="ExternalInput")
    d_we = nc.dram_tensor("we", [E, D, D], f32r, kind="ExternalInput")
    d_be = nc.dram_tensor("be", [E, D], f32r, kind="ExternalInput")

    d_out = nc.dram_tensor("out", [NS, D], f32, kind="ExternalOutput")
    d_gat = nc.dram_tensor("gat", [NS, E], f32, kind="ExternalOutput")

    xt_st = d_xt.ap()
    out_t4 = d_out.ap().rearrange("(s j p) d -> s j p d", p=128, j=ST)
    noise_st = d_noise.ap().rearrange("(s j p) e -> s p j e", p=128, j=ST)
    gat_st_d = d_gat.ap().rearrange("(s j p) e -> s p j e", p=128, j=ST)

    with TileContext(nc) as tc:
        with tc.tile_pool(name="const", bufs=1) as cpool, \
             tc.tile_pool(name="wpool", bufs=1) as wpool, \
             tc.tile_pool(name="sb", bufs=3) as sb, \
             tc.tile_pool(name="st", bufs=3) as stp, \
             tc.tile_pool(name="ps", bufs=2, space="PSUM") as ps, \
             tc.tile_pool(name="pse", bufs=6, space="PSUM") as pse_pool:

            wrn_sb = cpool.tile([128, 4, 2 * E], f32, tag="wrn")
            nc.sync.dma_start(wrn_sb[:], d_wrn.ap().rearrange("(c p) n -> p c n", p=128))
            we_sb = wpool.tile([128, E, 4, D], f32r, tag="we")
            nc.sync.dma_start(we_sb[:], d_we.ap().rearrange("e (c p) n -> p e c n", p=128))
            if use_rbias:
                brn_sb = cpool.tile([1, 2 * E], f32, tag="brn")
                nc.sync.dma_start(brn_sb[:], d_brn.ap())
                ones_f = cpool.tile([1, 128], f32, tag="onesf")
                nc.vector.memset(ones_f[:], 1.0)
            if use_bias:
                ones_r = cpool.tile([1, 128], f32r, tag="ones")
                nc.vector.memset(ones_r[:], 1.0)
                be_sb = cpool.tile([1, E, D], f32r, tag="be")
                nc.sync.dma_start(be_sb[:], d_be.ap()[None])

            pending = []

            def emit_experts(s, xtr, gat):
                for j in range(ST):
                    ea, eb = schedule[s * ST + j]
                    ps_a = pse_pool.tile([128, D], f32, tag="pse")
                    ps_b = pse_pool.tile([128, D], f32, tag="pse")
                    for c in range(4):
                        nc.tensor.matmul(ps_a[:], xtr[:, j, c], we_sb[:, ea, c],
                                         start=(c == 0), stop=(c == 3 and not use_bias))
                        nc.tensor.matmul(ps_b[:], xtr[:, j, c], we_sb[:, eb, c],
                                         start=(c == 0), stop=(c == 3 and not use_bias))
                    if use_bias:
                        nc.tensor.matmul(ps_a[:], ones_r[:], be_sb[:, ea], start=False, stop=True)
                        nc.tensor.matmul(ps_b[:], ones_r[:], be_sb[:, eb], start=False, stop=True)
                    acc = sb.tile([128, D], f32, tag="acc")
                    nc.scalar.activation(acc[:], ps_a[:], AF.Copy, scale=gat[:, j, ea:ea + 1])
                    nc.vector.scalar_tensor_tensor(acc[:], ps_b[:], gat[:, j, eb:eb + 1], acc[:],
                                                   OP.mult, OP.add)
                    nc.sync.dma_start(out_t4[s, j], acc[:])

            for s in range(n_st):
                xt32 = stp.tile([128, ST, 4, 128], f32, tag="xt32")
                xtr = stp.tile([128, ST, 4, 128], f32r, tag="xtr")
                lg = stp.tile([128, ST, 2 * E], f32, tag="lg")
                nc.sync.dma_start(xt32[:].rearrange("p j c n -> p (j c n)"), xt_st[:, s])
                nc.vector.tensor_copy(xtr[:], xt32[:])

                for j in range(ST):
                    psr = ps.tile([128, 2 * E], f32, tag="psr")
                    for c in range(4):
                        nc.tensor.matmul(psr[:], xt32[:, j, c], wrn_sb[:, c],
                                         start=(c == 0), stop=(c == 3 and not use_rbias))
                    if use_rbias:
                        nc.tensor.matmul(psr[:], ones_f[:], brn_sb[:], start=False, stop=True)
                    nc.vector.tensor_copy(lg[:, j], psr[:])

                # ---- routing vector stage on [128, ST, 8] ----
                noise_t = sb.tile([128, ST, E], f32, tag="noise")
                nc.sync.dma_start(noise_t[:], noise_st[s])
                logit = lg[:, :, 0:E]
                nlog = lg[:, :, E:2 * E]
                shp = (128, ST, E)
                t_a = sb.tile(list(shp), f32, tag="va")
                nc.scalar.activation(t_a[:], nlog, AF.Abs)
                nc.scalar.activation(t_a[:], t_a[:], AF.Exp, scale=-1.0)
                nc.scalar.activation(t_a[:], t_a[:], AF.Ln, bias=1.0)
                nc.vector.scalar_tensor_tensor(t_a[:], nlog, 0.0, t_a[:], OP.max, OP.add)
                noisy = sb.tile(list(shp), f32, tag="vn")
                nc.vector.tensor_tensor(noisy[:], noise_t[:], t_a[:], OP.mult)
                nc.vector.tensor_tensor(noisy[:], noisy[:], logit, OP.add)
                m1 = sb.tile([128, ST, 1], f32, tag="vm1")
                nc.vector.tensor_reduce(m1[:], noisy[:], mybir.AxisListType.X, OP.max)
                eq = sb.tile(list(shp), f32, tag="veq")
                nc.vector.tensor_tensor(eq[:], noisy[:], m1[:].to_broadcast(shp), OP.is_equal)
                nc.vector.scalar_tensor_tensor(eq[:], eq[:], -1e30, noisy[:], OP.mult, OP.add)
                m2 = sb.tile([128, ST, 1], f32, tag="vm2")
                nc.vector.tensor_reduce(m2[:], eq[:], mybir.AxisListType.X, OP.max)
                mask2 = sb.tile(list(shp), f32, tag="vmk")
                nc.vector.tensor_tensor(mask2[:], noisy[:], m2[:].to_broadcast(shp), OP.is_ge)
                sh_t = sb.tile(list(shp), f32, tag="vsh")
                nc.vector.tensor_tensor(sh_t[:], noisy[:], m1[:].to_broadcast(shp), OP.subtract)
                nc.scalar.activation(sh_t[:], sh_t[:], AF.Exp)
                gat = stp.tile([128, ST, E], f32, tag="gat")
                nc.vector.tensor_tensor(gat[:], sh_t[:], mask2[:], OP.mult)
                den = sb.tile([128, ST, 1], f32, tag="vdn")
                nc.vector.tensor_reduce(den[:], gat[:], mybir.AxisListType.X, OP.add)
                nc.vector.reciprocal(den[:], den[:])
                nc.vector.tensor_tensor(gat[:], gat[:], den[:].to_broadcast(shp), OP.mult)
                nc.sync.dma_start(gat_st_d[s], gat[:])

                pending.append((s, xtr, gat))
                if len(pending) > 1:
                    emit_experts(*pending.pop(0))
            for args in pending:
                emit_experts(*args)

    nc.compile()
    _CACHE[key] = nc
    return nc


def _compute_noise():
    import jax
    cpu = jax.devices("cpu")[0]
    with jax.default_device(cpu):
        import jax.numpy as jnp
        key = jax.random.key(1234)
        return np.asarray(jax.random.normal(key, (B_FULL, E), dtype=jnp.float32))


def _route_host(x, Wr, br, Wn, bn, noise):
    """Replicate the reference routing decisions (fp32 numpy)."""
    logits = x @ Wr + br
    nl = x @ Wn + bn
    noisy = logits + noise * np.logaddexp(nl, 0.0).astype(np.float32)
    order = np.argsort(-noisy, axis=1, kind="stable")
    e1, e2 = order[:, 0].astype(np.int32), order[:, 1].astype(np.int32)
    a = np.minimum(e1, e2)
    b = np.maximum(e1, e2)
    return a * 8 + b


def prepare(x, Wr, br, Wn, bn, We, be):
    x = np.ascontiguousarray(np.asarray(x, dtype=np.float32))
    Wr = np.asarray(Wr, dtype=np.float32)
    br = np.asarray(br, dtype=np.float32)
    Wn = np.asarray(Wn, dtype=np.float32)
    bn = np.asarray(bn, dtype=np.float32)
    We = np.ascontiguousarray(np.asarray(We, dtype=np.float32))
    be = np.ascontiguousarray(np.asarray(be, dtype=np.float32))

    noise = _compute_noise()
    gid = _route_host(x, Wr, br, Wn, bn, noise)

    groups = []           # (gid, token_array)
    for g in range(64):
        idxs = np.nonzero(gid == g)[0]
        if len(idxs):
            groups.append((g, idxs))

    caps = []
    for g, idxs in groups:
        per_core_max = -(-len(idxs) // N_CORES)
        caps.append(-(-(per_core_max + SLACK) // 128) * 128)
    NT = sum(caps) // 128
    schedule = []
    for (g, _), cap in zip(groups, caps):
        schedule.extend([(g // 8, g % 8)] * (cap // 128))
    while len(schedule) % ST:
        schedule.append((0, 0))
    schedule = tuple(schedule)
    NS = len(schedule) * 128

    wrn = np.ascontiguousarray(np.concatenate([Wr, Wn], axis=1))
    brn = np.concatenate([br, bn])[None, :]
    use_bias = bool(np.any(be != 0.0))
    use_rbias = bool(np.any(brn != 0.0))

    nc = _build(schedule, use_bias, use_rbias)

    in_maps = []
    slot_maps = []        # per core: (slot_rows, token_ids)
    for c in range(N_CORES):
        x_c = np.zeros((NS, D), np.float32)
        n_c = np.zeros((NS, E), np.float32)
        rows = []
        toks = []
        off = 0
        for (g, idxs), cap in zip(groups, caps):
            sub = idxs[c::N_CORES]
            rows.append(np.arange(off, off + len(sub)))
            toks.append(sub)
            off += cap
        rows = np.concatenate(rows)
        toks = np.concatenate(toks)
        x_c[rows] = x[toks]
        n_c[rows] = noise[toks]
        n_st_h = NS // 512
        xt_c = np.ascontiguousarray(
            x_c.T.reshape(4, 128, n_st_h, ST, 128).transpose(1, 2, 3, 0, 4)
            .reshape(128, n_st_h, ST * 4 * 128))
        slot_maps.append((rows, toks))
        in_maps.append({"xt": xt_c, "noise": n_c, "wrn": wrn, "brn": brn,
                        "we": We, "be": be})

    return nc, in_maps, slot_maps


def collect(res, slot_maps):
    updates = np.empty((B_FULL, D), np.float32)
    gating = np.empty((B_FULL, E), np.float32)
    for c in range(N_CORES):
        rows, toks = slot_maps[c]
        updates[toks] = res.results[c]["out"][rows]
        gating[toks] = res.results[c]["gat"][rows]
    return updates, gating


def kernel(x, Wr, br, Wn, bn, We, be):
    nc, in_maps, slot_maps = prepare(x, Wr, br, Wn, bn, We, be)
    res = run_bass_kernel_spmd(nc, in_maps, core_ids=list(range(N_CORES)))
    return collect(res, slot_maps)


if __name__ == "__main__":
    print("smoke build...")
    sched = []
    for g in range(28):
        a = 0
        while (a + 1) * 8 - ((a + 1) * (a + 2)) // 2 <= g:
            a += 1
        sched.append((0, 1))
    _build(tuple((i % 7, (i % 7) + 1) for i in range(28)), False, False)
    print("built ok")
